# revision 2
# baseline (speedup 1.0000x reference)
"""Discounted cumsum (B,H,S,D)=(8,16,4096,128), gamma per head, scan along S.

Strategy: batch-parallel across 8 NeuronCores (1 batch each, all 16 heads).
Host pre-transposes x to [H, D, S] fp16 so the scan runs along the FREE axis
with d on partitions (D=128 exactly). Per head, one DVE tensor_tensor_scan
  state = gamma * state + x[:, t]      (state fp32, operands fp16)
does the whole recurrence; DMA is fully contiguous (8KB runs per partition),
so fp16 halves HBM traffic at full DMA bandwidth. Output y' [H, D, S] fp16 is
transposed back + upcast on the host.
"""
import sys

sys.path.insert(0, "/opt/trn_rl_repo")
import numpy as np

B, H, S, D = 8, 16, 4096, 128

_CACHE = {}


def _build(repeat=1):
    import contextlib

    import concourse.bacc as bacc
    import concourse.tile as tile
    from concourse import mybir

    f16 = mybir.dt.float16

    nc = bacc.Bacc("TRN2", target_bir_lowering=False, debug=False)

    x_in = nc.declare_dram_parameter("x", [H, D, S], f16, isOutput=False)
    g_in = nc.declare_dram_parameter("g", [D, H], f16, isOutput=False)
    y_out = nc.declare_dram_parameter("y", [H, D, S], f16, isOutput=True)

    with tile.TileContext(nc) as tc:
        with (
            tc.tile_pool(name="const", bufs=1) as const_pool,
            tc.tile_pool(name="xp", bufs=3) as x_pool,
            tc.tile_pool(name="yp", bufs=3) as y_pool,
        ):
            g_sb = const_pool.tile([D, H], f16)
            nc.sync.dma_start(out=g_sb[:], in_=g_in[:])

            loop = tc.For_i(0, repeat, 1) if repeat > 1 else contextlib.nullcontext()
            with loop:
                for h in range(H):
                    xt = x_pool.tile([D, S], f16, name=f"xt{h}", tag="xt")
                    nc.sync.dma_start(out=xt[:], in_=x_in[h])
                    yt = y_pool.tile([D, S], f16, name=f"yt{h}", tag="yt")
                    nc.vector.tensor_tensor_scan(
                        out=yt[:],
                        data0=g_sb[:, h : h + 1].broadcast_to([D, S]),
                        data1=xt[:],
                        initial=0.0,
                        op0=mybir.AluOpType.mult,
                        op1=mybir.AluOpType.add,
                    )
                    nc.scalar.dma_start(out=y_out[h], in_=yt[:])

    nc.compile()
    return nc


def _make_in_maps(tensor, gamma):
    """Full inputs -> per-core input maps (host-side shard + layout)."""
    x16 = np.asarray(tensor, dtype=np.float16)
    g_bc = np.tile(np.asarray(gamma, dtype=np.float16)[None, :], (D, 1))
    return [
        {
            "x": np.ascontiguousarray(x16[c].transpose(0, 2, 1)),
            "g": g_bc,
        }
        for c in range(B)
    ]


def _postprocess(y_parts):
    """Per-core y' [H, D, S] fp16 -> full [B, H, S, D] f32."""
    return np.stack(
        [y_parts[c].transpose(0, 2, 1).astype(np.float32) for c in range(B)], axis=0
    )


def _fast_callable(nc):
    """Cached jitted shard_map callable (avoids per-call retrace)."""
    import jax
    from jax.experimental.shard_map import shard_map
    from jax.sharding import Mesh, NamedSharding, PartitionSpec
    from concourse import bass2jax, mybir

    bass2jax.install_neuronx_cc_hook()
    partition_name = nc.partition_id_tensor.name if nc.partition_id_tensor else None
    in_names, out_names, out_avals, zero_outs = [], [], [], []
    for alloc in nc.m.functions[0].allocations:
        if not isinstance(alloc, mybir.MemoryLocationSet):
            continue
        name = alloc.memorylocations[0].name
        if alloc.kind == "ExternalInput":
            if name != partition_name:
                in_names.append(name)
        elif alloc.kind == "ExternalOutput":
            shape = tuple(alloc.tensor_shape)
            dtype = mybir.dt.np(alloc.dtype)
            out_avals.append(jax.core.ShapedArray(shape, dtype))
            out_names.append(name)
            zero_outs.append(np.zeros(shape, dtype))
    n_params = len(in_names)
    all_in = list(in_names) + list(out_names)
    if partition_name is not None:
        all_in.append(partition_name)

    def _body(*args):
        operands = list(args)
        if partition_name is not None:
            operands.append(bass2jax.partition_id_tensor())
        return tuple(
            bass2jax._bass_exec_p.bind(
                *operands,
                out_avals=tuple(out_avals),
                in_names=tuple(all_in),
                out_names=tuple(out_names),
                lowering_input_output_aliases=(),
                sim_require_finite=True,
                sim_require_nnan=True,
                nc=nc,
            )
        )

    devices = jax.devices()[:B]
    mesh = Mesh(np.asarray(devices), ("core",))
    specs = (PartitionSpec("core"),)
    f = jax.jit(
        shard_map(
            _body,
            mesh=mesh,
            in_specs=specs * (n_params + len(out_names)),
            out_specs=specs * len(out_names),
            check_rep=False,
        ),
        keep_unused=True,
    )
    sharding = NamedSharding(mesh, PartitionSpec("core"))
    dev_zero = [
        jax.device_put(np.zeros((B * z.shape[0], *z.shape[1:]), z.dtype), sharding)
        for z in zero_outs
    ]
    return f, in_names, out_names, out_avals, sharding, dev_zero


def _run_fast(nc, in_maps):
    import jax

    if "fast" not in _CACHE:
        _CACHE["fast"] = _fast_callable(nc)
    f, in_names, out_names, out_avals, sharding, dev_zero = _CACHE["fast"]
    concat_in = [
        jax.device_put(
            np.concatenate([np.asarray(m[nm]) for m in in_maps], axis=0), sharding
        )
        for nm in in_names
    ]
    outs = f(*concat_in, *dev_zero)
    return [
        {
            nm: np.asarray(outs[i]).reshape(B, *out_avals[i].shape)[c]
            for i, nm in enumerate(out_names)
        }
        for c in range(B)
    ]


def _run(tensor, gamma, trace=False, repeat=1):
    from concourse.bass_utils import run_bass_kernel_spmd

    key = f"nc{repeat}"
    if key not in _CACHE:
        _CACHE[key] = _build(repeat)
    nc = _CACHE[key]

    in_maps = _make_in_maps(tensor, gamma)
    if repeat == 1 and not trace:
        try:
            results = _run_fast(nc, in_maps)
            y = _postprocess([results[c]["y"] for c in range(B)])
            return y, None
        except Exception:
            pass  # fall back to the reference path below
    res = run_bass_kernel_spmd(nc, in_maps, core_ids=list(range(B)), trace=trace)
    y = _postprocess([res.results[c]["y"] for c in range(B)])
    return y, res


def kernel(tensor, gamma):
    try:
        y, _ = _run(tensor, gamma)
    except Exception:
        # transient device/pool errors: clear cached state and retry once
        _CACHE.clear()
        y, _ = _run(tensor, gamma)
    return y
